# revision 1
# baseline (speedup 1.0000x reference)
"""ActionRelationEncoder kernel for 8 Trainium2 NeuronCores.

Pure data parallelism over batch (B=64 -> 8 shards of 8 samples), all
weights replicated on every core, per spec sharding hint. Each core runs
the fused GNN-message-passing block (v-transform -> 2 steps of
2-direction graph self-attention with geometric bias) on its batch shard;
results are gathered back to the full [64, 128, 1024] output.
"""

import numpy as np
import jax
import jax.numpy as jnp

# dims (hardcoded per problem spec)
B, N, NG, H = 64, 128, 64, 16
VD, QD, OD, PD = 2048, 1024, 1024, 64
DG = OD // H
DIRS, STEPS = 2, 2
EPS = 1e-6
NCORES = 8

_cache = {}


def _attn(self_feat, pos_emb, bias_scalar, Wq, bq, Wk, bk, Wp, bp, Wout, bout):
    b = self_feat.shape[0]
    kv = self_feat[:, :NG]
    qh = (self_feat @ Wq.T + bq).reshape(b, N, H, DG)
    kh = (kv @ Wk.T + bk).reshape(b, NG, H, DG)
    aff = jnp.einsum('bnhd,bmhd->bnhm', qh, kh) / jnp.sqrt(jnp.float32(DG))
    pos_w = jax.nn.relu(
        jnp.einsum('bnmp,hp->bnhm', pos_emb, Wp) + bp[None, None, :, None]
    )
    aff = aff + jnp.log(jnp.maximum(pos_w, EPS)) + bias_scalar
    att = jax.nn.softmax(aff, axis=-1)
    # out-projection applied to values first: kvW[b,h,m,g] then att @ kvW.
    # Algebraically identical to einsum('bnhm,bmd->bnhd') + grouped conv,
    # but 3.5x fewer FLOPs.
    kvW = jnp.einsum('bmd,hgd->bhmg', kv, Wout)
    out = jnp.einsum('bnhm,bhmg->bnhg', att, kvW)
    return out.reshape(b, N, OD) + bout


def _gat(v_cat_q, pos_emb, Ws, bs, Wb, bb, Wq, bq, Wk, bk, Wp, bp, Wout, bout):
    self_feat = v_cat_q @ Ws.T + bs
    bias_scalar = Wb[0, 0] + bb[0]
    out = self_feat
    for d in range(DIRS):
        out = out + _attn(self_feat, pos_emb, bias_scalar,
                          Wq[d], bq[d], Wk[d], bk[d], Wp[d], bp[d],
                          Wout[d], bout[d])
    return jax.nn.relu(out)


def _forward(v, position_embedding, q, Wv, bv, Ws, bs, Wb, bb,
             Wq, bq, Wk, bk, Wp, bp, Wout, bout):
    act_v = jax.nn.relu(v @ Wv.T + bv)
    for _ in range(STEPS):
        mask = jnp.sum(act_v, axis=-1, keepdims=True) != 0
        q_exp = jnp.where(mask, q[:, None, :], jnp.float32(0))
        v_cat_q = jnp.concatenate([act_v, q_exp], axis=-1)
        rel = _gat(v_cat_q, position_embedding, Ws, bs, Wb, bb,
                   Wq, bq, Wk, bk, Wp, bp, Wout, bout)
        act_v = act_v + rel
    return act_v


def _get_pmapped():
    if 'fn' not in _cache:
        devs = jax.devices()[:NCORES]
        _cache['fn'] = jax.pmap(
            _forward,
            axis_name='cores',
            devices=devs,
            in_axes=(0, 0, 0) + (None,) * 14,
        )
    return _cache['fn']


def kernel(**inputs) -> np.ndarray:
    v = np.asarray(inputs['v'], np.float32)
    pos = np.asarray(inputs['position_embedding'], np.float32)
    q = np.asarray(inputs['q'], np.float32)
    weights = [np.asarray(inputs[k], np.float32) for k in
               ('Wv', 'bv', 'Ws', 'bs', 'Wb', 'bb', 'Wq', 'bq',
                'Wk', 'bk', 'Wp', 'bp', 'Wout', 'bout')]

    shard = B // NCORES  # 8 samples per core
    v_sh = v.reshape(NCORES, shard, N, VD)
    pos_sh = pos.reshape(NCORES, shard, N, NG, PD)
    q_sh = q.reshape(NCORES, shard, QD)

    fn = _get_pmapped()
    out = fn(v_sh, pos_sh, q_sh, *weights)
    out = np.asarray(out, np.float32).reshape(B, N, OD)
    return out


if __name__ == '__main__':
    rng = np.random.default_rng(0)
    ins = {
        'v': rng.standard_normal((B, N, VD), np.float32),
        'position_embedding': rng.random((B, N, NG, PD), np.float32),
        'q': rng.standard_normal((B, QD), np.float32),
        'Wv': 0.02 * rng.standard_normal((OD, VD), np.float32),
        'bv': np.zeros(OD, np.float32),
        'Ws': 0.02 * rng.standard_normal((OD, OD + QD), np.float32),
        'bs': np.zeros(OD, np.float32),
        'Wb': 0.02 * rng.standard_normal((1, 1), np.float32),
        'bb': np.zeros(1, np.float32),
        'Wq': 0.02 * rng.standard_normal((DIRS, OD, OD), np.float32),
        'bq': np.zeros((DIRS, OD), np.float32),
        'Wk': 0.02 * rng.standard_normal((DIRS, OD, OD), np.float32),
        'bk': np.zeros((DIRS, OD), np.float32),
        'Wp': 0.02 * rng.standard_normal((DIRS, H, PD), np.float32),
        'bp': np.zeros((DIRS, H), np.float32),
        'Wout': 0.02 * rng.standard_normal((DIRS, H, DG, OD), np.float32),
        'bout': np.zeros((DIRS, OD), np.float32),
    }
    out = kernel(**ins)
    print('kernel output', out.shape, out.dtype, float(np.abs(out).mean()))
